# revision 14
# baseline (speedup 1.0000x reference)
"""Multi-head attention (B=4, T=2048, D=1024, H=16) on 8 trn2 cores.

Tensor-parallel over heads: core c owns heads [2c, 2c+1] (= output
columns [128c, 128c+128) of the per-head projection space). Each core:
  1. projects full q/k/v against its 128-column weight slices,
     producing qh^T / kh^T (cols on partitions) and vh (natural layout,
     with a ones-column appended per head for the softmax denominator)
  2. per (batch, head): scores S^T = (w_q-slice scaled by 1/8 on host),
     exp on ScalarE, PV + sum-of-exp in one M=65 matmul, normalization
     via a K=1 broadcast matmul + vector multiply
  3. output projection against its w_o row slice -> per-core partial
     [B, T, D]; host sums partials over cores and adds b_o.

Inputs q/k/v are passed host-transposed ([B, D, T]) so no on-device
transposes are needed anywhere.
"""

import numpy as np

B, T, D, H = 4, 2048, 1024, 16
N_CORES = 8
HPC = H // N_CORES  # heads per core = 2
ND = D // H  # head dim = 64
COLS = HPC * ND  # 128 projection columns per core
DCH = D // 128  # 8 contraction chunks
NQ = T // 512  # 4 tq chunks of 512
NTK = T // 128  # 16 tk tiles of 128
VW = ND + 1  # 65: head cols + ones column
SG = 2  # tk tiles per score/exp group

_STATE = {}


def _split_multi_waits(bir_json: bytes) -> bytes:
    """The pinned walrus rejects >1 sync-wait command per instruction.
    Hoist extra waits into standalone EventSemaphore instructions emitted
    just before the instruction on the same engine/queue stream."""
    import orjson

    d = orjson.loads(bir_json)

    def fix_list(ins_list):
        out = []
        for ins in ins_list:
            si = ins.get("sync_info")
            waits = (si or {}).get("on_wait") or []
            if len(waits) > 1:
                for i, w in enumerate(waits[:-1]):
                    out.append(
                        {
                            "debug": ins.get("debug", 0),
                            "engine": ins["engine"],
                            "ins": [],
                            "outs": [],
                            "name": f"{ins['name']}_xw{i}",
                            "opcode": "EventSemaphore",
                            "sync_info": {"on_update": [], "on_wait": [w]},
                        }
                    )
                si["on_wait"] = [waits[-1]]
            out.append(ins)
        return out

    def rec(obj):
        if isinstance(obj, dict):
            for key, v in obj.items():
                if key == "instructions" and isinstance(v, list):
                    obj[key] = fix_list(v)
                else:
                    rec(v)
        elif isinstance(obj, list):
            for v in obj:
                rec(v)

    rec(d.get("functions"))
    return orjson.dumps(d)


def _install_bir_wait_splitter():
    if _STATE.get("splitter_installed"):
        return
    from concourse import bass2jax, bass_utils

    orig = bass_utils.compile_bir_kernel

    def wrapped(bir_json, tmpdir, neff_name="file.neff"):
        return orig(_split_multi_waits(bir_json), tmpdir, neff_name=neff_name)

    bass2jax.compile_bir_kernel = wrapped
    _STATE["splitter_installed"] = True


def _build(mm_dtype_name="float32r"):
    import concourse.bass as bass
    import concourse.mybir as mybir
    from concourse.tile import TileContext

    f32 = mybir.dt.float32
    mf = getattr(mybir.dt, mm_dtype_name)  # dtype of matmul operands
    AF = mybir.ActivationFunctionType

    nc = bass.Bass("TRN2", target_bir_lowering=False, debug=False, num_devices=N_CORES)

    qT = nc.dram_tensor("qT", [B, D, T], mf, kind="ExternalInput")
    kT = nc.dram_tensor("kT", [B, D, T], mf, kind="ExternalInput")
    vT = nc.dram_tensor("vT", [B, D, T], mf, kind="ExternalInput")
    wq = nc.dram_tensor("wq", [D, COLS], mf, kind="ExternalInput")
    wk = nc.dram_tensor("wk", [D, COLS], mf, kind="ExternalInput")
    wv = nc.dram_tensor("wv", [D, COLS], mf, kind="ExternalInput")
    wo = nc.dram_tensor("wo", [COLS, D], mf, kind="ExternalInput")
    bq = nc.dram_tensor("bq", [COLS], f32, kind="ExternalInput")
    bk = nc.dram_tensor("bk", [COLS], f32, kind="ExternalInput")
    bvb = nc.dram_tensor("bvb", [128, COLS], f32, kind="ExternalInput")
    y = nc.dram_tensor("y", [B, T, D], f32, kind="ExternalOutput")

    def mm(out, lhsT, rhs, start, stop):
        nc.tensor.matmul(out, lhsT, rhs, start=start, stop=stop)

    with TileContext(nc) as tc:
        with (
            tc.tile_pool(name="consts", bufs=1) as consts,
            tc.tile_pool(name="xin", bufs=3) as xin,
            tc.tile_pool(name="proj", bufs=2) as proj,
            tc.tile_pool(name="exps", bufs=4) as exps,
            tc.tile_pool(name="ysb", bufs=4) as ysb,
            tc.tile_pool(name="small", bufs=4) as small,
            tc.tile_pool(name="ps_big", bufs=2, space="PSUM") as ps_big,
            tc.tile_pool(name="ps_o", bufs=2, space="PSUM") as ps_o,
            tc.tile_pool(name="ps_med", bufs=2, space="PSUM") as ps_med,
        ):
            # ---- persistent weights / consts ----
            wq_sb = consts.tile([128, D], mf, tag="wq")
            wk_sb = consts.tile([128, D], mf, tag="wk")
            wv_sb = consts.tile([128, D], mf, tag="wv")
            wo_sb = consts.tile([128, D], mf, tag="wo")
            for w_dram, w_sb in ((wq, wq_sb), (wk, wk_sb), (wv, wv_sb)):
                nc.sync.dma_start(
                    w_sb.rearrange("p (dc c) -> p dc c", dc=DCH),
                    w_dram.ap().rearrange("(dc p) c -> p dc c", p=128),
                )
            nc.sync.dma_start(wo_sb[:], wo.ap())
            bq_sb = consts.tile([COLS, 1], f32, tag="bq")
            bk_sb = consts.tile([COLS, 1], f32, tag="bk")
            nc.sync.dma_start(bq_sb[:], bq.ap().rearrange("(p one) -> p one", one=1))
            nc.sync.dma_start(bk_sb[:], bk.ap().rearrange("(p one) -> p one", one=1))
            bvb_sb = consts.tile([128, COLS], f32, tag="bvb")
            nc.sync.dma_start(bvb_sb[:], bvb.ap())
            ones_sb = consts.tile([1, ND], f32, tag="ones")
            nc.gpsimd.memset(ones_sb[:], 1.0)
            ones_col = consts.tile([128, NTK * HPC], f32, tag="ones_col")
            nc.gpsimd.memset(ones_col[:], 1.0)

            for b in range(B):
                # ---- projections ----
                # qh^T / kh^T: [cols(128) part, T free]
                qhT = proj.tile([128, T], mf, tag="qhT")
                khT = proj.tile([128, T], mf, tag="khT")
                for src, w_sb, b_sb, dst in (
                    (qT, wq_sb, bq_sb, qhT),
                    (kT, wk_sb, bk_sb, khT),
                ):
                    for c in range(NQ):
                        x_in = xin.tile([128, DCH * 512], mf, tag="xin")
                        nc.sync.dma_start(
                            x_in.rearrange("p (dc n) -> p dc n", dc=DCH),
                            src.ap()[b].rearrange("(dc p) t -> p dc t", p=128)[
                                :, :, c * 512 : (c + 1) * 512
                            ],
                        )
                        ps = ps_med.tile([128, 512], f32, tag="ps_med")
                        for dc in range(DCH):
                            mm(
                                ps[:],
                                w_sb[:, dc * 128 : (dc + 1) * 128],
                                x_in[:, dc * 512 : (dc + 1) * 512],
                                start=(dc == 0),
                                stop=(dc == DCH - 1),
                            )
                        nc.vector.tensor_scalar_add(
                            dst[:, c * 512 : (c + 1) * 512], ps[:], b_sb[:]
                        )

                # vh: [tk part, (head cols | 1)*HPC free] with ones columns
                vh = proj.tile([128, NTK * VW * HPC], mf, tag="vh")
                # ones columns (softmax denominator) at offset ND of every
                # VW-wide strip; gpsimd memset can't write f32r, so copy
                # from an f32 const via DVE (which rounds to f32r)
                nc.vector.tensor_copy(
                    vh.rearrange("p (i c) -> p i c", c=VW)[:, :, ND : ND + 1],
                    ones_col[:].unsqueeze(-1),
                )
                for c in range(NQ):
                    v_in = xin.tile([128, DCH * 512], mf, tag="xin")
                    nc.sync.dma_start(
                        v_in.rearrange("p (dc n) -> p dc n", dc=DCH),
                        vT.ap()[b].rearrange("(dc p) t -> p dc t", p=128)[
                            :, :, c * 512 : (c + 1) * 512
                        ],
                    )
                    for i in range(4):
                        t = c * 4 + i
                        ps = ps_med.tile([128, COLS], f32, tag="ps_med")
                        for dc in range(DCH):
                            mm(
                                ps[:],
                                v_in[:, dc * 512 + i * 128 : dc * 512 + (i + 1) * 128],
                                wv_sb[:, dc * 128 : (dc + 1) * 128],
                                start=(dc == 0),
                                stop=(dc == DCH - 1),
                            )
                        nc.vector.tensor_add(
                            vh[:, t * 2 * VW : t * 2 * VW + ND],
                            ps[:, 0:ND],
                            bvb_sb[:, 0:ND],
                        )
                        nc.vector.tensor_add(
                            vh[:, t * 2 * VW + VW : t * 2 * VW + VW + ND],
                            ps[:, ND:COLS],
                            bvb_sb[:, ND:COLS],
                        )

                # ---- attention + output accumulation ----
                oT = proj.tile([128, T], mf, tag="oT")
                for h in range(HPC):
                    hp = h * ND
                    for c in range(NQ):
                        po = ps_o.tile([VW, 512], f32, tag="ps_o")
                        for g in range(NTK // SG):
                            ps = ps_big.tile([128, SG * 512], f32, tag="ps_big")
                            for j in range(SG):
                                t = g * SG + j
                                mm(
                                    ps[:, j * 512 : (j + 1) * 512],
                                    khT[hp : hp + ND, t * 128 : (t + 1) * 128],
                                    qhT[hp : hp + ND, c * 512 : (c + 1) * 512],
                                    start=True,
                                    stop=True,
                                )
                            es = exps.tile([128, SG * 512], mf, tag="exps")
                            nc.scalar.activation(es[:], ps[:], AF.Exp)
                            for j in range(SG):
                                t = g * SG + j
                                mm(
                                    po[:],
                                    vh[:, (t * 2 + h) * VW : (t * 2 + h) * VW + VW],
                                    es[:, j * 512 : (j + 1) * 512],
                                    start=(t == 0),
                                    stop=(t == NTK - 1),
                                )
                        rec = small.tile([1, 512], f32, tag="rec")
                        nc.vector.reciprocal(rec[:], po[ND : ND + 1, :])
                        pb = ps_med.tile([ND, 512], f32, tag="ps_med")
                        nc.tensor.matmul(
                            pb[:], ones_sb[:, 0:ND], rec[:], start=True, stop=True
                        )
                        nc.vector.tensor_copy(
                            oT[hp : hp + ND, c * 512 : (c + 1) * 512], po[0:ND, :]
                        )
                        nc.vector.tensor_mul(
                            oT[hp : hp + ND, c * 512 : (c + 1) * 512],
                            oT[hp : hp + ND, c * 512 : (c + 1) * 512],
                            pb[:],
                        )

                # ---- output projection ----
                for tt in range(NTK):
                    y_sb = ysb.tile([128, D], f32, tag="ysb")
                    for nch in range(2):
                        ps = ps_med.tile([128, 512], f32, tag="ps_med")
                        mm(
                            ps[:],
                            oT[:, tt * 128 : (tt + 1) * 128],
                            wo_sb[:, nch * 512 : (nch + 1) * 512],
                            start=True,
                            stop=True,
                        )
                        nc.vector.tensor_copy(
                            y_sb[:, nch * 512 : (nch + 1) * 512], ps[:]
                        )
                    nc.sync.dma_start(
                        y.ap()[b, tt * 128 : (tt + 1) * 128, :], y_sb[:]
                    )

    return nc


def _get_nc():
    if "nc" not in _STATE:
        _STATE["nc"] = _build()
    return _STATE["nc"]


def kernel(q, k, v, w_q, b_q, w_k, b_k, w_v, b_v, w_o, b_o):
    from concourse import bass_utils

    _install_bir_wait_splitter()

    q = np.asarray(q, np.float32)
    k = np.asarray(k, np.float32)
    v = np.asarray(v, np.float32)
    qT = np.ascontiguousarray(np.transpose(q, (0, 2, 1)))
    kT = np.ascontiguousarray(np.transpose(k, (0, 2, 1)))
    vT = np.ascontiguousarray(np.transpose(v, (0, 2, 1)))

    scale = 1.0 / np.sqrt(np.float32(ND))
    in_maps = []
    for c in range(N_CORES):
        s = slice(c * COLS, (c + 1) * COLS)
        in_maps.append(
            {
                "qT": qT,
                "kT": kT,
                "vT": vT,
                "wq": np.ascontiguousarray(w_q[:, s] * scale),
                "wk": np.ascontiguousarray(w_k[:, s]),
                "wv": np.ascontiguousarray(w_v[:, s]),
                "wo": np.ascontiguousarray(w_o[s, :]),
                "bq": np.ascontiguousarray(b_q[s] * scale),
                "bk": np.ascontiguousarray(b_k[s]),
                "bvb": np.ascontiguousarray(
                    np.broadcast_to(b_v[s][None, :], (128, COLS))
                ),
            }
        )

    nc = _get_nc()
    res = bass_utils.run_bass_kernel_spmd(nc, in_maps, core_ids=list(range(N_CORES)))
    _STATE["last_results"] = res
    out = np.zeros((B, T, D), np.float64)
    for c in range(N_CORES):
        out += res.results[c]["y"].astype(np.float64)
    out += np.asarray(b_o, np.float64)[None, None, :]
    return out.astype(np.float32)


# revision 18
# speedup vs baseline: 1.8647x; 1.8647x over previous
"""Multi-head attention (B=4, T=2048, D=1024, H=16) on 8 trn2 cores.

Tensor-parallel over heads: core c owns heads [2c, 2c+1] (= output
columns [128c, 128c+128) of the per-head projection space). Each core:
  1. projects full q/k/v against its 128-column weight slices,
     producing qh^T / kh^T (cols on partitions) and vh (natural layout,
     with a ones-column appended per head for the softmax denominator)
  2. per (batch, head): scores S^T = (w_q-slice scaled by 1/8 on host),
     exp on ScalarE, PV + sum-of-exp in one M=65 matmul, normalization
     via a K=1 broadcast matmul + vector multiply
  3. output projection against its w_o row slice -> per-core partial
     [B, T, D]; host sums partials over cores and adds b_o.

Inputs q/k/v are passed host-transposed ([B, D, T]) so no on-device
transposes are needed anywhere.
"""

import numpy as np

B, T, D, H = 4, 2048, 1024, 16
N_CORES = 8
HPC = H // N_CORES  # heads per core = 2
ND = D // H  # head dim = 64
COLS = HPC * ND  # 128 projection columns per core
DCH = D // 128  # 8 contraction chunks
NQ = T // 512  # 4 tq chunks of 512
NTK = T // 128  # 16 tk tiles of 128
VW = ND + 1  # 65: head cols + ones column
SG = 2  # tk tiles per score/exp group

_STATE = {}


def _split_multi_waits(bir_json: bytes) -> bytes:
    """The pinned walrus rejects >1 sync-wait command per instruction.
    Hoist extra waits into standalone EventSemaphore instructions emitted
    just before the instruction on the same engine/queue stream."""
    import orjson

    d = orjson.loads(bir_json)

    def fix_list(ins_list):
        out = []
        for ins in ins_list:
            si = ins.get("sync_info")
            waits = (si or {}).get("on_wait") or []
            if len(waits) > 1:
                for i, w in enumerate(waits[:-1]):
                    out.append(
                        {
                            "debug": ins.get("debug", 0),
                            "engine": ins["engine"],
                            "ins": [],
                            "outs": [],
                            "name": f"{ins['name']}_xw{i}",
                            "opcode": "EventSemaphore",
                            "sync_info": {"on_update": [], "on_wait": [w]},
                        }
                    )
                si["on_wait"] = [waits[-1]]
            out.append(ins)
        return out

    def rec(obj):
        if isinstance(obj, dict):
            for key, v in obj.items():
                if key == "instructions" and isinstance(v, list):
                    obj[key] = fix_list(v)
                else:
                    rec(v)
        elif isinstance(obj, list):
            for v in obj:
                rec(v)

    rec(d.get("functions"))
    return orjson.dumps(d)


def _install_bir_wait_splitter():
    if _STATE.get("splitter_installed"):
        return
    from concourse import bass2jax, bass_utils

    orig = bass_utils.compile_bir_kernel

    def wrapped(bir_json, tmpdir, neff_name="file.neff"):
        return orig(_split_multi_waits(bir_json), tmpdir, neff_name=neff_name)

    bass2jax.compile_bir_kernel = wrapped
    _STATE["splitter_installed"] = True


def _build(mm_dtype_name="bfloat16"):
    import concourse.bass as bass
    import concourse.mybir as mybir
    from concourse.tile import TileContext

    f32 = mybir.dt.float32
    mf = getattr(mybir.dt, mm_dtype_name)  # dtype of matmul operands
    AF = mybir.ActivationFunctionType

    nc = bass.Bass("TRN2", target_bir_lowering=False, debug=False, num_devices=N_CORES)

    qT = nc.dram_tensor("qT", [B, D, T], mf, kind="ExternalInput")
    kT = nc.dram_tensor("kT", [B, D, T], mf, kind="ExternalInput")
    vT = nc.dram_tensor("vT", [B, D, T], mf, kind="ExternalInput")
    wq = nc.dram_tensor("wq", [D, COLS], mf, kind="ExternalInput")
    wk = nc.dram_tensor("wk", [D, COLS], mf, kind="ExternalInput")
    wv = nc.dram_tensor("wv", [D, COLS], mf, kind="ExternalInput")
    wo = nc.dram_tensor("wo", [COLS, D], mf, kind="ExternalInput")
    bq = nc.dram_tensor("bq", [COLS], f32, kind="ExternalInput")
    bk = nc.dram_tensor("bk", [COLS], f32, kind="ExternalInput")
    bvb = nc.dram_tensor("bvb", [128, COLS], f32, kind="ExternalInput")
    sel = nc.dram_tensor("sel", [HPC * NQ, HPC * NQ * ND], mf, kind="ExternalInput")
    y = nc.dram_tensor("y", [B, T, D], f32, kind="ExternalOutput")

    def mm(out, lhsT, rhs, start, stop):
        nc.tensor.matmul(out, lhsT, rhs, start=start, stop=stop)

    with TileContext(nc) as tc:
        with (
            tc.tile_pool(name="consts", bufs=1) as consts,
            tc.tile_pool(name="xin", bufs=3) as xin,
            tc.tile_pool(name="proj", bufs=2) as proj,
            tc.tile_pool(name="exps", bufs=4) as exps,
            tc.tile_pool(name="ysb", bufs=4) as ysb,
            tc.tile_pool(name="small", bufs=4) as small,
            tc.tile_pool(name="ps_big", bufs=2, space="PSUM") as ps_big,
            tc.tile_pool(name="ps_o", bufs=2, space="PSUM") as ps_o,
            tc.tile_pool(name="ps_med", bufs=2, space="PSUM") as ps_med,
        ):
            # ---- persistent weights / consts ----
            wq_sb = consts.tile([128, D], mf, tag="wq")
            wk_sb = consts.tile([128, D], mf, tag="wk")
            wv_sb = consts.tile([128, D], mf, tag="wv")
            wo_sb = consts.tile([128, D], mf, tag="wo")
            for w_dram, w_sb in ((wq, wq_sb), (wk, wk_sb), (wv, wv_sb)):
                nc.sync.dma_start(
                    w_sb.rearrange("p (dc c) -> p dc c", dc=DCH),
                    w_dram.ap().rearrange("(dc p) c -> p dc c", p=128),
                )
            nc.sync.dma_start(wo_sb[:], wo.ap())
            bq_sb = consts.tile([COLS, 1], f32, tag="bq")
            bk_sb = consts.tile([COLS, 1], f32, tag="bk")
            nc.sync.dma_start(bq_sb[:], bq.ap().rearrange("(p one) -> p one", one=1))
            nc.sync.dma_start(bk_sb[:], bk.ap().rearrange("(p one) -> p one", one=1))
            bvb_sb = consts.tile([128, COLS], f32, tag="bvb")
            nc.sync.dma_start(bvb_sb[:], bvb.ap())
            sel_sb = consts.tile([HPC * NQ, HPC * NQ * ND], mf, tag="sel")
            nc.sync.dma_start(sel_sb[:], sel.ap())
            ones_col = consts.tile([128, NTK * HPC], f32, tag="ones_col")
            nc.gpsimd.memset(ones_col[:], 1.0)

            for b in range(B):
                # ---- projections ----
                # qh^T / kh^T: [cols(128) part, T free]
                qhT = proj.tile([128, T], mf, tag="qhT")
                khT = proj.tile([128, T], mf, tag="khT")
                for src, w_sb, b_sb, dst in (
                    (qT, wq_sb, bq_sb, qhT),
                    (kT, wk_sb, bk_sb, khT),
                ):
                    for c in range(NQ):
                        x_in = xin.tile([128, DCH * 512], mf, tag="xin")
                        nc.sync.dma_start(
                            x_in.rearrange("p (dc n) -> p dc n", dc=DCH),
                            src.ap()[b].rearrange("(dc p) t -> p dc t", p=128)[
                                :, :, c * 512 : (c + 1) * 512
                            ],
                        )
                        ps = ps_med.tile([128, 512], f32, tag="ps_med")
                        for dc in range(DCH):
                            mm(
                                ps[:],
                                w_sb[:, dc * 128 : (dc + 1) * 128],
                                x_in[:, dc * 512 : (dc + 1) * 512],
                                start=(dc == 0),
                                stop=(dc == DCH - 1),
                            )
                        nc.vector.tensor_scalar_add(
                            dst[:, c * 512 : (c + 1) * 512], ps[:], b_sb[:]
                        )

                # vh: [tk part, (head cols | 1)*HPC free] with ones columns
                vh = proj.tile([128, NTK * VW * HPC], mf, tag="vh")
                # ones columns (softmax denominator) at offset ND of every
                # VW-wide strip; gpsimd memset can't write f32r, so copy
                # from an f32 const via DVE (which rounds to f32r)
                nc.vector.tensor_copy(
                    vh.rearrange("p (i c) -> p i c", c=VW)[:, :, ND : ND + 1],
                    ones_col[:].unsqueeze(-1),
                )
                for c in range(NQ):
                    v_in = xin.tile([128, DCH * 512], mf, tag="xin")
                    nc.sync.dma_start(
                        v_in.rearrange("p (dc n) -> p dc n", dc=DCH),
                        vT.ap()[b].rearrange("(dc p) t -> p dc t", p=128)[
                            :, :, c * 512 : (c + 1) * 512
                        ],
                    )
                    for i in range(4):
                        t = c * 4 + i
                        ps = ps_med.tile([128, COLS], f32, tag="ps_med")
                        for dc in range(DCH):
                            mm(
                                ps[:],
                                v_in[:, dc * 512 + i * 128 : dc * 512 + (i + 1) * 128],
                                wv_sb[:, dc * 128 : (dc + 1) * 128],
                                start=(dc == 0),
                                stop=(dc == DCH - 1),
                            )
                        nc.vector.tensor_add(
                            vh[:, t * 2 * VW : t * 2 * VW + ND],
                            ps[:, 0:ND],
                            bvb_sb[:, 0:ND],
                        )
                        nc.vector.tensor_add(
                            vh[:, t * 2 * VW + VW : t * 2 * VW + VW + ND],
                            ps[:, ND:COLS],
                            bvb_sb[:, ND:COLS],
                        )

                # ---- attention + output accumulation ----
                oT = proj.tile([128, T], mf, tag="oT")
                sums = small.tile([HPC * NQ, 512], f32, tag="sums")
                rec = small.tile([HPC * NQ, 512], f32, tag="rec")
                for h in range(HPC):
                    hp = h * ND
                    for c in range(NQ):
                        po = ps_o.tile([VW, 512], f32, tag="ps_o")
                        for g in range(NTK // SG):
                            ps = ps_big.tile([128, SG * 512], f32, tag="ps_big")
                            for j in range(SG):
                                t = g * SG + j
                                mm(
                                    ps[:, j * 512 : (j + 1) * 512],
                                    khT[hp : hp + ND, t * 128 : (t + 1) * 128],
                                    qhT[hp : hp + ND, c * 512 : (c + 1) * 512],
                                    start=True,
                                    stop=True,
                                )
                            es = exps.tile([128, SG * 512], mf, tag="exps")
                            nc.scalar.activation(es[:], ps[:], AF.Exp)
                            for j in range(SG):
                                t = g * SG + j
                                mm(
                                    po[:],
                                    vh[:, (t * 2 + h) * VW : (t * 2 + h) * VW + VW],
                                    es[:, j * 512 : (j + 1) * 512],
                                    start=(t == 0),
                                    stop=(t == NTK - 1),
                                )
                        i = h * NQ + c
                        nc.vector.tensor_copy(
                            oT[hp : hp + ND, c * 512 : (c + 1) * 512], po[0:ND, :]
                        )
                        # engines can't write partition base i; stage the sum
                        # row at partition 0 and DMA it into sums row i
                        stg = small.tile([1, 512], f32, tag="stg")
                        nc.scalar.copy(stg[:], po[ND : ND + 1, :])
                        nc.sync.dma_start(sums[i : i + 1, :], stg[:])
                # one reciprocal over all 8 (h, c) sum rows of this batch,
                # then broadcast row i across 64 partitions via a K=8 matmul
                # against a one-hot selector (base_partition must be 0)
                nc.vector.reciprocal(rec[:], sums[:])
                rec_mf = small.tile([HPC * NQ, 512], mf, tag="rec_mf")
                nc.vector.tensor_copy(rec_mf[:], rec[:])
                for h in range(HPC):
                    hp = h * ND
                    for c in range(NQ):
                        i = h * NQ + c
                        pb = ps_med.tile([ND, 512], f32, tag="ps_med")
                        nc.tensor.matmul(
                            pb[:],
                            sel_sb[:, i * ND : (i + 1) * ND],
                            rec_mf[:],
                            start=True,
                            stop=True,
                        )
                        nc.vector.tensor_mul(
                            oT[hp : hp + ND, c * 512 : (c + 1) * 512],
                            oT[hp : hp + ND, c * 512 : (c + 1) * 512],
                            pb[:],
                        )

                # ---- output projection ----
                for tt in range(NTK):
                    y_sb = ysb.tile([128, D], f32, tag="ysb")
                    for nch in range(2):
                        ps = ps_med.tile([128, 512], f32, tag="ps_med")
                        mm(
                            ps[:],
                            oT[:, tt * 128 : (tt + 1) * 128],
                            wo_sb[:, nch * 512 : (nch + 1) * 512],
                            start=True,
                            stop=True,
                        )
                        nc.vector.tensor_copy(
                            y_sb[:, nch * 512 : (nch + 1) * 512], ps[:]
                        )
                    nc.sync.dma_start(
                        y.ap()[b, tt * 128 : (tt + 1) * 128, :], y_sb[:]
                    )

    return nc


def _get_nc():
    if "nc" not in _STATE:
        _STATE["nc"] = _build(MM_DTYPE)
    return _STATE["nc"]


MM_DTYPE = "bfloat16"


def _mm_np_dtype():
    if MM_DTYPE == "bfloat16":
        import ml_dtypes

        return np.dtype(ml_dtypes.bfloat16)
    return np.dtype(np.float32)


def _sel_matrix(md):
    n = HPC * NQ
    sel = np.zeros((n, n * ND), np.float32)
    for i in range(n):
        sel[i, i * ND : (i + 1) * ND] = 1.0
    return sel.astype(md)


def kernel(q, k, v, w_q, b_q, w_k, b_k, w_v, b_v, w_o, b_o):
    from concourse import bass_utils

    _install_bir_wait_splitter()
    md = _mm_np_dtype()
    q = np.asarray(q, np.float32)
    k = np.asarray(k, np.float32)
    v = np.asarray(v, np.float32)
    qT = np.ascontiguousarray(np.transpose(q, (0, 2, 1))).astype(md)
    kT = np.ascontiguousarray(np.transpose(k, (0, 2, 1))).astype(md)
    vT = np.ascontiguousarray(np.transpose(v, (0, 2, 1))).astype(md)

    scale = 1.0 / np.sqrt(np.float32(ND))
    in_maps = []
    for c in range(N_CORES):
        s = slice(c * COLS, (c + 1) * COLS)
        in_maps.append(
            {
                "qT": qT,
                "kT": kT,
                "vT": vT,
                "wq": np.ascontiguousarray(w_q[:, s] * scale).astype(md),
                "wk": np.ascontiguousarray(w_k[:, s]).astype(md),
                "wv": np.ascontiguousarray(w_v[:, s]).astype(md),
                "wo": np.ascontiguousarray(w_o[s, :]).astype(md),
                "bq": np.ascontiguousarray(b_q[s] * scale),
                "bk": np.ascontiguousarray(b_k[s]),
                "bvb": np.ascontiguousarray(
                    np.broadcast_to(b_v[s][None, :], (128, COLS))
                ).astype(np.float32),
                "sel": _sel_matrix(md),
            }
        )

    nc = _get_nc()
    res = bass_utils.run_bass_kernel_spmd(nc, in_maps, core_ids=list(range(N_CORES)))
    _STATE["last_results"] = res
    out = np.zeros((B, T, D), np.float64)
    for c in range(N_CORES):
        out += res.results[c]["y"].astype(np.float64)
    out += np.asarray(b_o, np.float64)[None, None, :]
    return out.astype(np.float32)
